# revision 23
# baseline (speedup 1.0000x reference)
"""Trainium2 kernel for nn_AdaFastFoodMergedModel.

FastFood transform: y = SCALE * Sel . H . diag(G) . Pi . H . diag(B) . x
(H = 4096-point orthonormal Walsh-Hadamard, Pi = random permutation,
Sel = row subset of size 1228).

Strategy: everything right of `x` is a fixed linear operator built from the
small inputs (B, G, Pi, row_idx), so fold it on the host into one dense
matrix W [4096, 1228] (bf16) and run y = x @ W on the TensorEngine.

The host also pre-arranges x per core into xt[rt][p, kc, r] bf16 (feature-
on-partition), so the device runs a pure matmul pipeline: no on-device
casts, no DMA transposes.  Per core (rows sharded 8192/8 = 1024):
  - W loads as 8 group tiles [128, 4, 1228] on the scalar ring; xt row
    tiles (1MB each, contiguous per partition) on the gpsimd ring; y
    output on the sync ring -- three balanced DMA queues, 23.4MB total.
  - phase 1 computes row-tiles 0+1 interleaved GROUP-major across 6 psum
    banks, paced by the arriving W stream; row-tiles 2..7 then run
    back-to-back with W SBUF-resident (kc-major, one LDWEIGHTS feeding
    all 3 sel-chunk matmuls).
  - psum evacuation split across DVE and ACT.
No cross-core communication (data parallel over rows).
"""

import math
import sys

import numpy as np

sys.path.insert(0, "/opt/trn_rl_repo")

import ml_dtypes

ROWS, D = 8192, 4096
M = 1228
SCALE = math.sqrt(D / M)
N_CORES = 8
SHARD = ROWS // N_CORES  # 1024
P = 128
KC = D // P  # 32 contraction chunks
RT = SHARD // P  # 8 row tiles per core
SEL_CHUNKS = [(0, 512), (512, 512), (1024, 204)]  # 1228 = 512+512+204
WG = 8  # W group loads
KPG = KC // WG  # 4 kc per group
CH = 4  # column chunks per row tile
CW = D // CH  # 1024
KCC = KC // CH  # 8 k-chunks per column-chunk
WARMUP_MM = 0

# set by test harness to collect a profile
TRACE = False
LAST = {}

_CACHE = {}


def _fwht_cols(a: np.ndarray) -> np.ndarray:
    """Orthonormal FWHT along axis 0 (Sylvester/natural order)."""
    n = a.shape[0]
    x = a.copy()
    h = 1
    while h < n:
        x = x.reshape(n // (2 * h), 2, h, -1)
        lo = x[:, 0]
        hi = x[:, 1]
        x = np.stack((lo + hi, lo - hi), axis=1).reshape(n, -1)
        h *= 2
    return x * (1.0 / math.sqrt(n))


def _build_w(B, G, Pi, row_idx) -> np.ndarray:
    """W such that y = x @ W  (float32)."""
    S = np.zeros((D, M), dtype=np.float64)
    S[row_idx, np.arange(M)] = 1.0  # Sel^T
    A = _fwht_cols(S)  # H .
    A = A * G[:, None].astype(np.float64)  # diag(G) .
    A2 = np.empty_like(A)
    A2[Pi] = A  # Pi^T .
    A2 = _fwht_cols(A2)  # H .
    W = SCALE * (B[:, None].astype(np.float64) * A2)  # diag(B) .
    return W.astype(np.float32)


def _install_ntff_shim():
    """The image's antenv lacks axon_hooks; provide it so
    run_bass_kernel_spmd(trace=True) can collect an NTFF profile."""
    import types

    try:
        import antenv.axon_hooks  # noqa: F401

        return
    except ImportError:
        pass
    try:
        from trn_agent_boot.trn_boot import _ntff_profile_via_ctypes

        hook = _ntff_profile_via_ctypes("/opt/axon/libaxon_pjrt.so")
    except Exception:
        hook = None
    mod = types.ModuleType("antenv.axon_hooks")
    mod.get_axon_ntff_profile_hook = lambda: hook
    mod.set_axon_ntff_profile_hook = lambda h: None
    sys.modules["antenv.axon_hooks"] = mod


def _build_bass():
    import concourse.bass as bass
    import concourse.bacc as bacc
    import concourse.mybir as mybir
    from concourse import tile

    f32 = mybir.dt.float32
    bf16 = mybir.dt.bfloat16

    nc = bacc.Bacc("TRN2", target_bir_lowering=False, debug=False)
    xt_in = [
        nc.declare_dram_parameter(f"xt{rt}", [P, KC, P], bf16, isOutput=False)
        for rt in range(RT)
    ]
    # W pre-arranged on host to the SBUF layout [p, kc, m] so each DMA is
    # contiguous per partition
    w_in = nc.declare_dram_parameter("w", [P, KC, M], bf16, isOutput=False)
    out = nc.declare_dram_parameter("out", [SHARD, M], f32, isOutput=True)

    with tile.TileContext(nc) as tc:
        with (
            tc.tile_pool(name="const", bufs=1) as const_pool,
            tc.tile_pool(name="xT", bufs=1) as xT_pool,
            tc.tile_pool(name="y", bufs=2) as y_pool,
            tc.tile_pool(name="psy", bufs=1, space=bass.MemorySpace.PSUM) as psy_pool,
        ):
            w_g = [
                const_pool.tile([P, KPG, M], bf16, tag=f"w{g}", name=f"w{g}")
                for g in range(WG)
            ]
            for g in range(WG):
                nc.scalar.dma_start(w_g[g][:], w_in[:, g * KPG : (g + 1) * KPG, :])

            def emit_load(rt):
                xT = xT_pool.tile(
                    [P, KC, P], bf16, tag=f"xT{rt % 4}", name=f"xT{rt % 4}"
                )
                nc.gpsimd.dma_start(xT[:], xt_in[rt][:, :, :])
                return xT

            def emit_evac_out(rt, psys):
                y_sb = y_pool.tile([P, M], f32, tag="y", name="y")
                nc.vector.tensor_copy(y_sb[:, 0:512], psys[0][:])
                nc.sync.dma_start(out[rt * P : (rt + 1) * P, 0:512], y_sb[:, 0:512])
                nc.scalar.copy(y_sb[:, 512:1024], psys[1][:])
                nc.sync.dma_start(
                    out[rt * P : (rt + 1) * P, 512:1024], y_sb[:, 512:1024]
                )
                nc.vector.tensor_copy(y_sb[:, 1024:1228], psys[2][:])
                nc.sync.dma_start(
                    out[rt * P : (rt + 1) * P, 1024:1228], y_sb[:, 1024:1228]
                )

            def new_psys(slot):
                return [
                    psy_pool.tile(
                        [P, sz], f32, tag=f"psy{slot}c{ci}", name=f"psy{slot}c{ci}"
                    )
                    for ci, (off, sz) in enumerate(SEL_CHUNKS)
                ]

            # PE p-state warm-up: the PE is idle until the first W group
            # lands (~19us); ~10us of full-width matmuls in that dead window
            # give the clock governor sustained activity to ramp MID->FULL
            # before phase 1 starts.  Uses the psy2c0 bank (first real use
            # is at group 2, ~28us).
            warm_l = const_pool.tile([P, 512], bf16, tag="warm_l", name="warm_l")
            nc.gpsimd.memset(warm_l[:], 0.0)
            warm_ps = psy_pool.tile([P, 512], f32, tag="psy2c0", name="warm_ps")
            for i in range(24):
                nc.tensor.matmul(
                    warm_ps[:],
                    warm_l[:, 0:P],
                    warm_l[:],
                    start=(i == 0),
                    stop=(i == 23),
                )

            xTs = {0: emit_load(0)}

            # --- phase 1: row tiles 0+1 fully, plus rt2's first two sel
            # chunks (2+3+3 = 8 psum banks), interleaved group-major and
            # paced by the W stream.  rt1 starts at group 1 and rt2 at
            # group 2 (accumulation order within a chain is free), so the
            # PE is never waiting on data that has not arrived yet.
            ps01 = {0: new_psys(0), 1: new_psys(1)}
            ps2 = [
                psy_pool.tile(
                    [P, SEL_CHUNKS[ci][1]], f32, tag=f"psy2c{ci}", name=f"psy2c{ci}"
                )
                for ci in range(2)
            ]

            def p1_mm(kc, first, last, pss, nchunk=3):
                g = kc // KPG
                rt = 0 if pss is ps01[0] else (1 if pss is ps01[1] else 2)
                for ci in range(nchunk):
                    off, sz = SEL_CHUNKS[ci]
                    nc.tensor.matmul(
                        pss[ci][:],
                        xTs[rt][:, kc, :],
                        w_g[g][:, kc % KPG, off : off + sz],
                        start=first,
                        stop=last,
                    )

            xTs[1] = emit_load(1)
            for kc in range(0, KPG):  # g0: rt0 solo
                p1_mm(kc, kc == 0, False, ps01[0])
            xTs[2] = emit_load(2)
            for kc in range(KPG, 2 * KPG):  # g1: rt0 + rt1
                p1_mm(kc, False, False, ps01[0])
                p1_mm(kc, kc == KPG, False, ps01[1])
            for g in range(2, WG):  # g2..7: all three
                if g == 4:
                    xTs[3] = emit_load(3)
                for kc in range(g * KPG, (g + 1) * KPG):
                    p1_mm(kc, False, kc == KC - 1, ps01[0])
                    p1_mm(kc, False, False, ps01[1])
                    p1_mm(kc, kc == 2 * KPG, False, ps2, 2)
            for kc in range(0, KPG):  # rt1 wraps to g0; rt2 continues
                p1_mm(kc, False, kc == KPG - 1, ps01[1])
                p1_mm(kc, False, False, ps2, 2)
            for kc in range(KPG, 2 * KPG):  # rt2 finishes with g1
                p1_mm(kc, False, kc == 2 * KPG - 1, ps2, 2)
            emit_evac_out(0, ps01[0])
            emit_evac_out(1, ps01[1])

            # --- phase 2: rt2 finishes its third sel chunk, then row
            # tiles 3..7 back-to-back, kc-major ---
            xTs[4] = emit_load(4)
            psc2 = psy_pool.tile(
                [P, SEL_CHUNKS[2][1]], f32, tag="psy0c2", name="psy0c2b"
            )
            off2, sz2 = SEL_CHUNKS[2]
            for kc in range(KC):
                nc.tensor.matmul(
                    psc2[:],
                    xTs[2][:, kc, :],
                    w_g[kc // KPG][:, kc % KPG, off2 : off2 + sz2],
                    start=(kc == 0),
                    stop=(kc == KC - 1),
                )
            emit_evac_out(2, [ps2[0], ps2[1], psc2])
            for rt in range(3, RT - 1):
                if rt + 2 < RT:
                    xTs[rt + 2] = emit_load(rt + 2)
                psys = new_psys(rt % 2)
                for kc in range(KC):
                    g = kc // KPG
                    for ci, (off, sz) in enumerate(SEL_CHUNKS):
                        nc.tensor.matmul(
                            psys[ci][:],
                            xTs[rt][:, kc, :],
                            w_g[g][:, kc % KPG, off : off + sz],
                            start=(kc == 0),
                            stop=(kc == KC - 1),
                        )
                emit_evac_out(rt, psys)

            # last row tile: chains run ci-serial so each chunk's evac and
            # output DMA overlap the remaining chains (shorter tail)
            rt = RT - 1
            psys = new_psys(rt % 2)
            y_sb = y_pool.tile([P, M], f32, tag="y", name="ylast")
            for ci, (off, sz) in enumerate(SEL_CHUNKS):
                for kc in range(KC):
                    nc.tensor.matmul(
                        psys[ci][:],
                        xTs[rt][:, kc, :],
                        w_g[kc // KPG][:, kc % KPG, off : off + sz],
                        start=(kc == 0),
                        stop=(kc == KC - 1),
                    )
                eng = nc.scalar if ci == 1 else nc.vector
                if ci == 1:
                    nc.scalar.copy(y_sb[:, off : off + sz], psys[ci][:])
                else:
                    nc.vector.tensor_copy(y_sb[:, off : off + sz], psys[ci][:])
                nc.sync.dma_start(
                    out[rt * P : (rt + 1) * P, off : off + sz],
                    y_sb[:, off : off + sz],
                )

    nc.compile()
    return nc


def kernel(x, B, G, Pi, row_idx):
    x = np.ascontiguousarray(np.asarray(x, dtype=np.float32))
    B = np.asarray(B, dtype=np.float32)
    G = np.asarray(G, dtype=np.float32)
    Pi = np.asarray(Pi, dtype=np.int32)
    row_idx = np.asarray(row_idx, dtype=np.int32)

    W = _build_w(B, G, Pi, row_idx).astype(ml_dtypes.bfloat16)
    # rearrange to SBUF layout [p, kc, m]: W[kc*128+p, m] -> Wp[p, kc, m]
    Wp = np.ascontiguousarray(W.reshape(KC, P, M).transpose(1, 0, 2))

    if "nc" not in _CACHE:
        _CACHE["nc"] = _build_bass()
    nc = _CACHE["nc"]

    if TRACE:
        _install_ntff_shim()

    from concourse.bass_utils import run_bass_kernel_spmd

    # host pre-arrangement: xt[core][rt][p, kc, r] = x[core*1024 + rt*128 + r,
    # kc*128 + p] in bf16 -- feature-on-partition, contiguous per partition
    xt = np.ascontiguousarray(
        x.reshape(N_CORES, RT, P, KC, P).transpose(0, 1, 4, 3, 2)
    ).astype(ml_dtypes.bfloat16)
    in_maps = []
    for i in range(N_CORES):
        m = {"w": Wp}
        for rt in range(RT):
            m[f"xt{rt}"] = np.ascontiguousarray(xt[i, rt])
        in_maps.append(m)

    res = run_bass_kernel_spmd(
        nc, in_maps, core_ids=list(range(N_CORES)), trace=TRACE
    )
    LAST["exec_time_ns"] = getattr(res, "exec_time_ns", None)
    LAST["results"] = res

    outs = [np.asarray(res.results[i]["out"]) for i in range(N_CORES)]
    return np.concatenate(outs, axis=0).astype(np.float32)


if __name__ == "__main__":
    rng = np.random.default_rng(0)
    x = rng.standard_normal((ROWS, D), dtype=np.float32)
    B = (rng.integers(0, 2, D) * 2 - 1).astype(np.float32)
    G = rng.standard_normal(D, dtype=np.float32)
    Pi = rng.permutation(D).astype(np.int32)
    row_idx = rng.permutation(D)[:M].astype(np.int32)
    y = kernel(x=x, B=B, G=G, Pi=Pi, row_idx=row_idx)
    print("out", y.shape, y.dtype)
